# revision 2
# baseline (speedup 1.0000x reference)
"""GNN message-passing kernel for Trainium2 (8 NeuronCores).

Computation (see problem reference):
    x  = h.reshape(N, V, D)
    y  = relu(A @ (x_v W1_v) + b1_v)   per view v     (A = segment-sum over edges)
    z  = relu(A @ (y_v W2)   + b2)     per view v
    out = z.reshape(N, V*H)

Key restructure: aggregation commutes with the per-view linear maps,
    A @ (x W) = (A @ x) W
so we aggregate raw features first (one gather per edge, 1536B rows) and apply
the small dense weights to the aggregated 128-node chunks.

Mapping to hardware:
  - dst nodes are bin-packed into 8 cores x 50 chunks x 128 slots, balancing
    per-bin edge counts so one uniform SPMD schedule fits every core.
  - per chunk, edges are gathered with bulk dma_gather (int16 indices; the
    gather table is split at row 32768 into lo/hi views to cover >32k rows).
  - scatter-into-nodes is a one-hot matmul: S[e, n] = (dst_rel[e] == n), built
    on-device with is_equal against an iota row; m_chunk = sum_t S_t^T @ X_t
    accumulated in PSUM.
  - features travel as packed bf16 (hi | lo) pairs: x ~= hi + lo with relative
    error ~2^-17, letting the PE run at bf16 rate (1 cycle/row vs 4 for fp32).
  - after round 1, per-core y shards are AllGathered in two table-aligned
    pieces (32768 = 8x4096 "lo" rows, 18432 = 8x2304 "hi" rows) so round-2 lo
    gathers only wait on the first collective.
"""

import sys

if '/opt/trn_rl_repo' not in sys.path:
    sys.path.insert(0, '/opt/trn_rl_repo')

import numpy as np
import ml_dtypes

import concourse.bacc as bacc
import concourse.bass as bass
import concourse.mybir as mybir
import concourse.tile as tile
from concourse import bass_utils
from concourse.masks import make_identity

P = 128
N_NODES = 50000
N_EDGES = 400000
V = 3
D = 128
F = V * D            # 384 fp32 feature width
EP = 2 * F           # 768 bf16 packed (hi | lo)
NCORES = 8
NCHUNK = 50          # chunks per core
SLOTS = NCHUNK * P   # 6400 slots per core
NFULL = NCORES * SLOTS  # 51200
SPLIT = 32768        # int16 gather-table row limit
CHUNK_A = 32         # chunks 0..31 feed the first (lo-table) AllGather
ROWS_A = CHUNK_A * P          # 4096 rows per core -> 32768 total
ROWS_B = SLOTS - ROWS_A       # 2304 rows per core -> 18432 total

_BUILD_CACHE = {}


def _pack_bins(w_lo, w_hi, cap_lo, cap_hi):
    """Assign each node to a (core, chunk) bin: 400 bins x 128 slots,
    balancing lo/hi edge counts under the given caps. Returns slot[N]."""
    nbins = NCORES * NCHUNK
    n = len(w_lo)
    bin_lo = np.zeros(nbins, np.int64)
    bin_hi = np.zeros(nbins, np.int64)
    bin_cnt = np.zeros(nbins, np.int64)
    bin_members = [[] for _ in range(nbins)]
    order = np.argsort(-(w_lo + w_hi), kind='stable')
    wl = w_lo.astype(np.int64)
    wh = w_hi.astype(np.int64)
    for node in order:
        l, h = wl[node], wh[node]
        feas = (bin_cnt < P) & (bin_lo + l <= cap_lo) & (bin_hi + h <= cap_hi)
        if not feas.any():
            feas = bin_cnt < P
        load = np.maximum((bin_lo + l) / cap_lo, (bin_hi + h) / cap_hi)
        load = np.where(feas, load, np.inf)
        b = int(np.argmin(load))
        bin_members[b].append(node)
        bin_lo[b] += l
        bin_hi[b] += h
        bin_cnt[b] += 1
    slot = np.full(n, -1, np.int64)
    for b in range(nbins):
        for i, node in enumerate(bin_members[b]):
            slot[node] = b * P + i
    t_lo = int(-(-bin_lo.max() // P))
    t_hi = int(-(-bin_hi.max() // P))
    return slot, t_lo, t_hi


def _edge_tables(key, dst_slot, t_lo, t_hi):
    """Build per-core gather index + dst_rel arrays for one round.

    key: per-edge gather-table row (round-1: src id; round-2: y-table row of
    src). Returns idx_lo [8,50,t_lo*128] i16, idx_hi [8,50,t_hi*128] i16
    (pad 0), rel [8,50,(t_lo+t_hi)*128] f32 (pad -1); lo tiles then hi.
    """
    cap_l, cap_h = t_lo * P, t_hi * P
    e_bin = dst_slot // P
    e_rel = (dst_slot % P).astype(np.float64)
    is_hi = key >= SPLIT

    idx_lo = np.zeros((NCORES, NCHUNK, cap_l), np.int16)
    idx_hi = np.zeros((NCORES, NCHUNK, cap_h), np.int16)
    rel = np.full((NCORES, NCHUNK, cap_l + cap_h), -1.0, np.float32)

    order = np.lexsort((key, is_hi, e_bin))
    sb = e_bin[order]
    sh = is_hi[order]
    sk = key[order]
    sr = e_rel[order]
    grp = sb * 2 + sh
    new = np.ones(len(grp), bool)
    new[1:] = grp[1:] != grp[:-1]
    idxs = np.arange(len(grp))
    start = np.maximum.accumulate(np.where(new, idxs, 0))
    pos = idxs - start

    lo_m = ~sh
    b_lo, p_lo = sb[lo_m], pos[lo_m]
    assert p_lo.max(initial=0) < cap_l, "lo stream overflow; bump t_lo"
    idx_lo[b_lo // NCHUNK, b_lo % NCHUNK, p_lo] = sk[lo_m].astype(np.int16)
    rel[b_lo // NCHUNK, b_lo % NCHUNK, p_lo] = sr[lo_m]

    b_hi, p_hi = sb[sh], pos[sh]
    assert p_hi.max(initial=0) < cap_h, "hi stream overflow; bump t_hi"
    idx_hi[b_hi // NCHUNK, b_hi % NCHUNK, p_hi] = (sk[sh] - SPLIT).astype(np.int16)
    rel[b_hi // NCHUNK, b_hi % NCHUNK, cap_l + p_hi] = sr[sh]
    return idx_lo, idx_hi, rel


def _idx_layout(idx):
    """[NCORES, NCHUNK, cnt] -> [NCORES, 128, NCHUNK*cnt//16] int16 in the
    dma_gather wrapped layout (16-partition wrap, replicated x8)."""
    nc_, nch, cnt = idx.shape
    a = idx.reshape(nc_, nch, cnt // 16, 16)
    a = a.transpose(0, 3, 1, 2)
    a = a.reshape(nc_, 16, nch * (cnt // 16))
    return np.tile(a, (1, 8, 1)).copy()


def _rel_layout(rel):
    """[NCORES, NCHUNK, T*128] -> [NCORES, 128, NCHUNK*T] f32; column
    (chunk*T + t) holds tile t's 128 dst_rel values."""
    nc_, nch, tot = rel.shape
    t = tot // P
    a = rel.reshape(nc_, nch, t, P)
    a = a.transpose(0, 3, 1, 2).reshape(nc_, P, nch * t)
    return np.ascontiguousarray(a)


def _pack_bf16(x):
    hi = x.astype(ml_dtypes.bfloat16)
    lo = (x - hi.astype(np.float32)).astype(ml_dtypes.bfloat16)
    return np.concatenate([hi, lo], axis=1)


def _build(t_lo1, t_hi1, t_lo2, t_hi2, zero_bias=False, sim_single=False,
           repeat=1, ablate=None):
    key = (t_lo1, t_hi1, t_lo2, t_hi2, zero_bias, sim_single, repeat, ablate)
    if key in _BUILD_CACHE:
        return _BUILD_CACHE[key]

    nc = bacc.Bacc("TRN2", target_bir_lowering=False, debug=False,
                   num_devices=1 if sim_single else NCORES)
    bf16 = mybir.dt.bfloat16
    f32 = mybir.dt.float32
    i16 = mybir.dt.int16

    h_pk = nc.dram_tensor("h_pk", [N_NODES, EP], bf16, kind="ExternalInput")
    w1 = nc.dram_tensor("w1", [V, D, D], f32, kind="ExternalInput")
    w2 = nc.dram_tensor("w2", [D, D], f32, kind="ExternalInput")
    b1r = nc.dram_tensor("b1r", [P, F], f32, kind="ExternalInput")
    b2r = nc.dram_tensor("b2r", [P, F], f32, kind="ExternalInput")
    iota_in = nc.dram_tensor("iota", [P, P], f32, kind="ExternalInput")
    idx1_lo = nc.dram_tensor("idx1_lo", [P, NCHUNK * t_lo1 * 8], i16, kind="ExternalInput")
    idx1_hi = nc.dram_tensor("idx1_hi", [P, NCHUNK * t_hi1 * 8], i16, kind="ExternalInput")
    idx2_lo = nc.dram_tensor("idx2_lo", [P, NCHUNK * t_lo2 * 8], i16, kind="ExternalInput")
    idx2_hi = nc.dram_tensor("idx2_hi", [P, NCHUNK * t_hi2 * 8], i16, kind="ExternalInput")
    rel1_in = nc.dram_tensor("rel1", [P, NCHUNK * (t_lo1 + t_hi1)], f32, kind="ExternalInput")
    rel2_in = nc.dram_tensor("rel2", [P, NCHUNK * (t_lo2 + t_hi2)], f32, kind="ExternalInput")
    z_out = nc.dram_tensor("z_out", [SLOTS, F], f32, kind="ExternalOutput")

    with tile.TileContext(nc) as tc:
        with (
            tc.tile_pool(name="const", bufs=1) as cpool,
            tc.tile_pool(name="glo", bufs=4) as glo_pool,
            tc.tile_pool(name="ghi", bufs=4) as ghi_pool,
            tc.tile_pool(name="work", bufs=3) as work,
            tc.tile_pool(name="sel", bufs=4) as sel_pool,
            tc.tile_pool(name="ps_m", bufs=2, space="PSUM") as ps_m,
            tc.tile_pool(name="ps_y", bufs=2, space="PSUM") as ps_y,
            tc.tile_pool(name="ps_t", bufs=2, space="PSUM") as ps_t,
            tc.tile_pool(name="dram", bufs=1, space="DRAM") as dram,
        ):
            # constants
            iota_t = cpool.tile([P, P], f32)
            nc.sync.dma_start(iota_t[:], iota_in[:])
            ident = cpool.tile([P, P], f32)
            make_identity(nc, ident[:])
            w1_t = cpool.tile([P, V * D], f32)
            nc.sync.dma_start(
                w1_t[:].rearrange("d (v h) -> d v h", v=V),
                w1[:].rearrange("v d h -> d v h"),
            )
            w2_t = cpool.tile([P, D], f32)
            nc.sync.dma_start(w2_t[:], w2[:])
            b1_t = cpool.tile([P, F], f32)
            nc.sync.dma_start(b1_t[:], b1r[:])
            b2_t = cpool.tile([P, F], f32)
            nc.sync.dma_start(b2_t[:], b2r[:])

            idx_tiles = {}
            for name, ten, tcount in (
                ("1lo", idx1_lo, t_lo1), ("1hi", idx1_hi, t_hi1),
                ("2lo", idx2_lo, t_lo2), ("2hi", idx2_hi, t_hi2),
            ):
                it = cpool.tile([P, NCHUNK * tcount * 8], i16, tag=f"idx{name}")
                nc.sync.dma_start(it[:], ten[:])
                idx_tiles[name] = it
            rel1_t = cpool.tile([P, NCHUNK * (t_lo1 + t_hi1)], f32)
            nc.sync.dma_start(rel1_t[:], rel1_in[:])
            rel2_t = cpool.tile([P, NCHUNK * (t_lo2 + t_hi2)], f32)
            nc.sync.dma_start(rel2_t[:], rel2_in[:])

            def do_round(rnd, t_lo, t_hi, idx_lo_t, idx_hi_t, rel_t,
                         table_lo, table_hi, ya, yb):
                ntile = t_lo + t_hi
                for c in range(NCHUNK):
                    if ablate != "compute":
                        g_lo = glo_pool.tile([P, t_lo * EP], bf16, tag="glo")
                        nc.gpsimd.dma_gather(
                            out_ap=g_lo[:].rearrange("p (t e) -> p t e", e=EP),
                            in_ap=table_lo,
                            idxs_ap=idx_lo_t[:, c * t_lo * 8:(c + 1) * t_lo * 8],
                            num_idxs=t_lo * P,
                            num_idxs_reg=t_lo * P,
                            elem_size=EP,
                        )
                        g_hi = ghi_pool.tile([P, t_hi * EP], bf16, tag="ghi")
                        nc.gpsimd.dma_gather(
                            out_ap=g_hi[:].rearrange("p (t e) -> p t e", e=EP),
                            in_ap=table_hi,
                            idxs_ap=idx_hi_t[:, c * t_hi * 8:(c + 1) * t_hi * 8],
                            num_idxs=t_hi * P,
                            num_idxs_reg=t_hi * P,
                            elem_size=EP,
                        )
                    else:
                        g_lo = glo_pool.tile([P, t_lo * EP], bf16, tag="glo")
                        g_hi = ghi_pool.tile([P, t_hi * EP], bf16, tag="ghi")
                    if ablate == "gather":
                        continue
                    m_ps = ps_m.tile([P, F], f32, tag="m")
                    for t in range(ntile):
                        s_t = sel_pool.tile([P, P], bf16, tag="s")
                        nc.vector.tensor_tensor(
                            out=s_t[:],
                            in0=rel_t[:, c * ntile + t: c * ntile + t + 1]
                                .to_broadcast([P, P]),
                            in1=iota_t[:],
                            op=mybir.AluOpType.is_equal,
                        )
                        g = g_lo if t < t_lo else g_hi
                        tt = t if t < t_lo else t - t_lo
                        nc.tensor.matmul(
                            out=m_ps[:], lhsT=s_t[:],
                            rhs=g[:, tt * EP: tt * EP + F],
                            start=(t == 0), stop=False,
                        )
                        nc.tensor.matmul(
                            out=m_ps[:], lhsT=s_t[:],
                            rhs=g[:, tt * EP + F: (tt + 1) * EP],
                            start=False, stop=(t == ntile - 1),
                        )
                    # transform chunk: y = relu(m @ W + b)
                    m_sb = work.tile([P, F], f32, tag="m_sb")
                    nc.scalar.copy(m_sb[:], m_ps[:])
                    y_ps = ps_y.tile([P, F], f32, tag="y")
                    for v in range(V):
                        pt = ps_t.tile([P, P], f32, tag="pt")
                        nc.tensor.transpose(
                            pt[:], m_sb[:, v * P:(v + 1) * P], ident[:])
                        mT = work.tile([P, P], f32, tag="mT")
                        nc.scalar.copy(mT[:], pt[:])
                        rhs = (w1_t[:, v * P:(v + 1) * P] if rnd == 1
                               else w2_t[:])
                        nc.tensor.matmul(
                            out=y_ps[:, v * P:(v + 1) * P], lhsT=mT[:],
                            rhs=rhs, start=True, stop=True,
                        )
                    y_sb = work.tile([P, F], f32, tag="y_sb")
                    if zero_bias:
                        nc.scalar.activation(
                            y_sb[:], y_ps[:], mybir.ActivationFunctionType.Relu)
                    else:
                        nc.vector.tensor_add(
                            y_sb[:], y_ps[:], b1_t[:] if rnd == 1 else b2_t[:])
                        nc.scalar.activation(
                            y_sb[:], y_sb[:], mybir.ActivationFunctionType.Relu)
                    if rnd == 1:
                        y_pk = work.tile([P, EP], bf16, tag="y_pk")
                        nc.scalar.copy(y_pk[:, :F], y_sb[:])
                        nc.vector.tensor_sub(y_pk[:, F:], y_sb[:], y_pk[:, :F])
                        if c < CHUNK_A:
                            nc.sync.dma_start(
                                ya[c * P:(c + 1) * P, :], y_pk[:])
                        else:
                            nc.sync.dma_start(
                                yb[(c - CHUNK_A) * P:(c - CHUNK_A + 1) * P, :],
                                y_pk[:])
                    else:
                        nc.sync.dma_start(
                            z_out[c * P:(c + 1) * P, :], y_sb[:])

            for _rep in range(repeat):
                y_sh_a = dram.tile([ROWS_A, EP], bf16, tag=f"ysa{_rep}")
                y_sh_b = dram.tile([ROWS_B, EP], bf16, tag=f"ysb{_rep}")
                y_full_a = dram.tile([NCORES * ROWS_A, EP], bf16,
                                     addr_space="Shared", tag=f"yfa{_rep}")
                y_full_b = dram.tile([NCORES * ROWS_B, EP], bf16,
                                     addr_space="Shared", tag=f"yfb{_rep}")
                do_round(1, t_lo1, t_hi1, idx_tiles["1lo"], idx_tiles["1hi"],
                         rel1_t, h_pk[:SPLIT], h_pk[SPLIT:], y_sh_a, y_sh_b)

                if not sim_single and ablate != "gather":
                    nc.gpsimd.collective_compute(
                        "AllGather", mybir.AluOpType.bypass,
                        replica_groups=[list(range(NCORES))],
                        ins=[y_sh_a[:]], outs=[y_full_a[:]],
                    )
                    nc.gpsimd.collective_compute(
                        "AllGather", mybir.AluOpType.bypass,
                        replica_groups=[list(range(NCORES))],
                        ins=[y_sh_b[:]], outs=[y_full_b[:]],
                    )

                do_round(2, t_lo2, t_hi2, idx_tiles["2lo"], idx_tiles["2hi"],
                         rel2_t, y_full_a[:], y_full_b[:], None, None)

    nc.compile()
    _BUILD_CACHE[key] = nc
    return nc


def prep_inputs(h, src, dst, W1, b1, W2, b2):
    h = np.asarray(h, np.float32)
    src = np.asarray(src).astype(np.int64)
    dst = np.asarray(dst).astype(np.int64)
    W1 = np.asarray(W1, np.float32)
    b1 = np.asarray(b1, np.float32)
    W2 = np.asarray(W2, np.float32)
    b2 = np.asarray(b2, np.float32)

    # ---- host prep: index tables (integer metadata only) ----
    # round 1: gather key = src id (table = packed h split at 32768)
    lo1 = src < SPLIT
    w_lo1 = np.bincount(dst[lo1], minlength=N_NODES)
    w_hi1 = np.bincount(dst[~lo1], minlength=N_NODES)
    p1, t_lo1, t_hi1 = _pack_bins(w_lo1, w_hi1, 6 * P, 3 * P)
    t_lo1, t_hi1 = max(t_lo1, 6), max(t_hi1, 3)

    # round 2: gather key = y-table row of src. The y table is AllGathered in
    # two pieces: rows [0, ROWS_A) of each core (concat -> 32768 rows = lo
    # table), rows [ROWS_A, SLOTS) (concat -> 18432 rows = hi table).
    core1 = p1 // SLOTS
    row1 = p1 % SLOTS
    ytab = np.where(row1 < ROWS_A,
                    core1 * ROWS_A + row1,
                    SPLIT + core1 * ROWS_B + (row1 - ROWS_A))
    key2 = ytab[src]
    lo2 = key2 < SPLIT
    w_lo2 = np.bincount(dst[lo2], minlength=N_NODES)
    w_hi2 = np.bincount(dst[~lo2], minlength=N_NODES)
    p2, t_lo2, t_hi2 = _pack_bins(w_lo2, w_hi2, 6 * P, 3 * P)
    t_lo2, t_hi2 = max(t_lo2, 6), max(t_hi2, 3)

    i1l, i1h, r1 = _edge_tables(src, p1[dst], t_lo1, t_hi1)
    i2l, i2h, r2 = _edge_tables(key2, p2[dst], t_lo2, t_hi2)

    i1l, i1h = _idx_layout(i1l), _idx_layout(i1h)
    i2l, i2h = _idx_layout(i2l), _idx_layout(i2h)
    r1, r2 = _rel_layout(r1), _rel_layout(r2)

    h_pk = _pack_bf16(h)
    b1_flat = b1.reshape(F)
    b2_flat = np.tile(b2, V)
    zero_bias = not (b1_flat.any() or b2_flat.any())
    b1_rep = np.broadcast_to(b1_flat, (P, F)).copy()
    b2_rep = np.broadcast_to(b2_flat, (P, F)).copy()
    iota = np.broadcast_to(np.arange(P, dtype=np.float32), (P, P)).copy()

    in_maps = []
    for c in range(NCORES):
        in_maps.append({
            "h_pk": h_pk, "w1": W1, "w2": W2, "b1r": b1_rep, "b2r": b2_rep,
            "iota": iota,
            "idx1_lo": i1l[c], "idx1_hi": i1h[c],
            "idx2_lo": i2l[c], "idx2_hi": i2h[c],
            "rel1": r1[c], "rel2": r2[c],
        })

    return {
        "in_maps": in_maps,
        "tvals": (t_lo1, t_hi1, t_lo2, t_hi2),
        "zero_bias": zero_bias,
        "p2": p2,
    }


LAST_RESULT = None


def kernel(h, src, dst, W1, b1, W2, b2, _trace=False, _tmpdir=None):
    global LAST_RESULT
    prep = prep_inputs(h, src, dst, W1, b1, W2, b2)
    nc = _build(*prep["tvals"], zero_bias=prep["zero_bias"])
    res = bass_utils.run_bass_kernel_spmd(
        nc, prep["in_maps"], core_ids=list(range(NCORES)),
        trace=_trace, tmpdir=_tmpdir,
    )
    LAST_RESULT = res
    z_full = np.concatenate([res.results[c]["z_out"] for c in range(NCORES)],
                            axis=0)
    return z_full[prep["p2"]].astype(np.float32)



# revision 4
# speedup vs baseline: 1.1531x; 1.1531x over previous
"""GNN message-passing kernel for Trainium2 (8 NeuronCores).

Computation (see problem reference):
    x  = h.reshape(N, V, D)
    y  = relu(A @ (x_v W1_v) + b1_v)   per view v     (A = segment-sum over edges)
    z  = relu(A @ (y_v W2)   + b2)     per view v
    out = z.reshape(N, V*H)

Key restructure: aggregation commutes with the per-view linear maps,
    A @ (x W) = (A @ x) W
so we aggregate raw features first (one gather per edge) and apply the small
dense weights to the aggregated 128-node chunks.

Mapping to hardware:
  - dst nodes are bin-packed into 8 cores x 50 chunks x 128 slots, balancing
    per-bin edge counts so one uniform SPMD schedule fits every core.
  - per chunk, edges are gathered with bulk dma_gather (int16 indices; the
    gather table is split at row 32768 into lo/hi views to cover >32k rows).
  - scatter-into-nodes is a one-hot matmul: S[e, n] = (dst_rel[e] == n), built
    on-device with is_equal against an iota row; m_chunk = sum_t S_t^T @ X_t
    accumulated in PSUM.
  - features travel as plain bf16 (the correctness gate is rel_err < 2e-2;
    bf16 keeps us ~1e-3) so gathers move 768B rows and the PE runs at bf16
    rate.
  - after round 1, per-core y shards are AllGathered in two table-aligned
    pieces (32768 = 8x4096 "lo" rows, 18432 = 8x2304 "hi" rows) so round-2 lo
    gathers only wait on the first collective.
"""

import sys

if '/opt/trn_rl_repo' not in sys.path:
    sys.path.insert(0, '/opt/trn_rl_repo')

import numpy as np
import ml_dtypes

import concourse.bacc as bacc
import concourse.bass as bass
import concourse.mybir as mybir
import concourse.tile as tile
from concourse import bass_utils
from concourse.masks import make_identity

P = 128
N_NODES = 50000
N_EDGES = 400000
V = 3
D = 128
F = V * D            # 384 feature width
EP = F               # bf16 row: 384 elems = 768 bytes
NCORES = 8
NCHUNK = 50          # chunks per core
SLOTS = NCHUNK * P   # 6400 slots per core
NFULL = NCORES * SLOTS  # 51200
SPLIT = 32768        # int16 gather-table row limit
CHUNK_A = 32         # chunks 0..31 feed the first (lo-table) AllGather
ROWS_A = CHUNK_A * P          # 4096 rows per core -> 32768 total
ROWS_B = SLOTS - ROWS_A       # 2304 rows per core -> 18432 total

_BUILD_CACHE = {}


def _pack_bins(w_lo, w_hi, cap_lo, cap_hi):
    """Assign each node to a (core, chunk) bin: 400 bins x 128 slots,
    balancing lo/hi edge counts under the given caps. Returns slot[N]."""
    nbins = NCORES * NCHUNK
    n = len(w_lo)
    bin_lo = np.zeros(nbins, np.int64)
    bin_hi = np.zeros(nbins, np.int64)
    bin_cnt = np.zeros(nbins, np.int64)
    bin_members = [[] for _ in range(nbins)]
    order = np.argsort(-(w_lo + w_hi), kind='stable')
    wl = w_lo.astype(np.int64)
    wh = w_hi.astype(np.int64)
    for node in order:
        l, h = wl[node], wh[node]
        feas = (bin_cnt < P) & (bin_lo + l <= cap_lo) & (bin_hi + h <= cap_hi)
        if not feas.any():
            feas = bin_cnt < P
        load = np.maximum((bin_lo + l) / cap_lo, (bin_hi + h) / cap_hi)
        load = np.where(feas, load, np.inf)
        b = int(np.argmin(load))
        bin_members[b].append(node)
        bin_lo[b] += l
        bin_hi[b] += h
        bin_cnt[b] += 1
    slot = np.full(n, -1, np.int64)
    for b in range(nbins):
        for i, node in enumerate(bin_members[b]):
            slot[node] = b * P + i
    t_lo = int(-(-bin_lo.max() // P))
    t_hi = int(-(-bin_hi.max() // P))
    return slot, t_lo, t_hi


def _edge_tables(key, dst_slot, t_lo, t_hi):
    """Build per-core gather index + dst_rel arrays for one round.

    key: per-edge gather-table row (round-1: src id; round-2: y-table row of
    src). Returns idx_lo [8,50,t_lo*128] i16, idx_hi [8,50,t_hi*128] i16
    (pad 0), rel [8,50,(t_lo+t_hi)*128] f32 (pad -1); lo tiles then hi.
    """
    cap_l, cap_h = t_lo * P, t_hi * P
    e_bin = dst_slot // P
    e_rel = (dst_slot % P).astype(np.float64)
    is_hi = key >= SPLIT

    idx_lo = np.zeros((NCORES, NCHUNK, cap_l), np.int16)
    idx_hi = np.zeros((NCORES, NCHUNK, cap_h), np.int16)
    rel = np.full((NCORES, NCHUNK, cap_l + cap_h), -1.0, np.float32)

    order = np.lexsort((key, is_hi, e_bin))
    sb = e_bin[order]
    sh = is_hi[order]
    sk = key[order]
    sr = e_rel[order]
    grp = sb * 2 + sh
    new = np.ones(len(grp), bool)
    new[1:] = grp[1:] != grp[:-1]
    idxs = np.arange(len(grp))
    start = np.maximum.accumulate(np.where(new, idxs, 0))
    pos = idxs - start

    lo_m = ~sh
    b_lo, p_lo = sb[lo_m], pos[lo_m]
    assert p_lo.max(initial=0) < cap_l, "lo stream overflow; bump t_lo"
    idx_lo[b_lo // NCHUNK, b_lo % NCHUNK, p_lo] = sk[lo_m].astype(np.int16)
    rel[b_lo // NCHUNK, b_lo % NCHUNK, p_lo] = sr[lo_m]

    b_hi, p_hi = sb[sh], pos[sh]
    assert p_hi.max(initial=0) < cap_h, "hi stream overflow; bump t_hi"
    idx_hi[b_hi // NCHUNK, b_hi % NCHUNK, p_hi] = (sk[sh] - SPLIT).astype(np.int16)
    rel[b_hi // NCHUNK, b_hi % NCHUNK, cap_l + p_hi] = sr[sh]
    return idx_lo, idx_hi, rel


def _idx_layout(idx):
    """[NCORES, NCHUNK, cnt] -> [NCORES, 128, NCHUNK*cnt//16] int16 in the
    dma_gather wrapped layout (16-partition wrap, replicated x8)."""
    nc_, nch, cnt = idx.shape
    a = idx.reshape(nc_, nch, cnt // 16, 16)
    a = a.transpose(0, 3, 1, 2)
    a = a.reshape(nc_, 16, nch * (cnt // 16))
    return np.tile(a, (1, 8, 1)).copy()


def _rel_layout(rel):
    """[NCORES, NCHUNK, T*128] -> [NCORES, 128, NCHUNK*T] f32; column
    (chunk*T + t) holds tile t's 128 dst_rel values."""
    nc_, nch, tot = rel.shape
    t = tot // P
    a = rel.reshape(nc_, nch, t, P)
    a = a.transpose(0, 3, 1, 2).reshape(nc_, P, nch * t)
    return np.ascontiguousarray(a)


def _build(t_lo1, t_hi1, t_lo2, t_hi2, zero_bias=False, sim_single=False,
           repeat=1, ablate=None):
    key = (t_lo1, t_hi1, t_lo2, t_hi2, zero_bias, sim_single, repeat, ablate)
    if key in _BUILD_CACHE:
        return _BUILD_CACHE[key]

    nc = bacc.Bacc("TRN2", target_bir_lowering=False, debug=False,
                   num_devices=1 if sim_single else NCORES)
    bf16 = mybir.dt.bfloat16
    f32 = mybir.dt.float32
    i16 = mybir.dt.int16

    h_pk = nc.dram_tensor("h_pk", [N_NODES, EP], bf16, kind="ExternalInput")
    w1 = nc.dram_tensor("w1", [V, D, D], bf16, kind="ExternalInput")
    w2 = nc.dram_tensor("w2", [D, D], bf16, kind="ExternalInput")
    b1r = nc.dram_tensor("b1r", [P, F], f32, kind="ExternalInput")
    b2r = nc.dram_tensor("b2r", [P, F], f32, kind="ExternalInput")
    iota_in = nc.dram_tensor("iota", [P, P], f32, kind="ExternalInput")
    idx1_lo = nc.dram_tensor("idx1_lo", [P, NCHUNK * t_lo1 * 8], i16, kind="ExternalInput")
    idx1_hi = nc.dram_tensor("idx1_hi", [P, NCHUNK * t_hi1 * 8], i16, kind="ExternalInput")
    idx2_lo = nc.dram_tensor("idx2_lo", [P, NCHUNK * t_lo2 * 8], i16, kind="ExternalInput")
    idx2_hi = nc.dram_tensor("idx2_hi", [P, NCHUNK * t_hi2 * 8], i16, kind="ExternalInput")
    rel1_in = nc.dram_tensor("rel1", [P, NCHUNK * (t_lo1 + t_hi1)], f32, kind="ExternalInput")
    rel2_in = nc.dram_tensor("rel2", [P, NCHUNK * (t_lo2 + t_hi2)], f32, kind="ExternalInput")
    z_out = nc.dram_tensor("z_out", [SLOTS, F], f32, kind="ExternalOutput")

    with tile.TileContext(nc) as tc:
        with (
            tc.tile_pool(name="const", bufs=1) as cpool,
            tc.tile_pool(name="glo", bufs=4) as glo_pool,
            tc.tile_pool(name="ghi", bufs=4) as ghi_pool,
            tc.tile_pool(name="work", bufs=3) as work,
            tc.tile_pool(name="sel", bufs=4) as sel_pool,
            tc.tile_pool(name="ps_m", bufs=2, space="PSUM") as ps_m,
            tc.tile_pool(name="ps_y", bufs=2, space="PSUM") as ps_y,
            tc.tile_pool(name="ps_t", bufs=2, space="PSUM") as ps_t,
            tc.tile_pool(name="dram", bufs=1, space="DRAM") as dram,
        ):
            # constants
            iota_t = cpool.tile([P, P], f32)
            nc.sync.dma_start(iota_t[:], iota_in[:])
            ident = cpool.tile([P, P], bf16)
            make_identity(nc, ident[:])
            w1_t = cpool.tile([P, V * D], bf16)
            nc.sync.dma_start(
                w1_t[:].rearrange("d (v h) -> d v h", v=V),
                w1[:].rearrange("v d h -> d v h"),
            )
            w2_t = cpool.tile([P, D], bf16)
            nc.sync.dma_start(w2_t[:], w2[:])
            b1_t = cpool.tile([P, F], f32)
            nc.sync.dma_start(b1_t[:], b1r[:])
            b2_t = cpool.tile([P, F], f32)
            nc.sync.dma_start(b2_t[:], b2r[:])

            idx_tiles = {}
            for name, ten, tcount in (
                ("1lo", idx1_lo, t_lo1), ("1hi", idx1_hi, t_hi1),
                ("2lo", idx2_lo, t_lo2), ("2hi", idx2_hi, t_hi2),
            ):
                it = cpool.tile([P, NCHUNK * tcount * 8], i16, tag=f"idx{name}")
                nc.sync.dma_start(it[:], ten[:])
                idx_tiles[name] = it
            rel1_t = cpool.tile([P, NCHUNK * (t_lo1 + t_hi1)], f32)
            nc.sync.dma_start(rel1_t[:], rel1_in[:])
            rel2_t = cpool.tile([P, NCHUNK * (t_lo2 + t_hi2)], f32)
            nc.sync.dma_start(rel2_t[:], rel2_in[:])

            def do_round(rnd, t_lo, t_hi, idx_lo_t, idx_hi_t, rel_t,
                         table_lo, table_hi, ya, yb):
                ntile = t_lo + t_hi
                for c in range(NCHUNK):
                    if ablate != "compute":
                        g_lo = glo_pool.tile([P, t_lo * EP], bf16, tag="glo")
                        nc.gpsimd.dma_gather(
                            out_ap=g_lo[:].rearrange("p (t e) -> p t e", e=EP),
                            in_ap=table_lo,
                            idxs_ap=idx_lo_t[:, c * t_lo * 8:(c + 1) * t_lo * 8],
                            num_idxs=t_lo * P,
                            num_idxs_reg=t_lo * P,
                            elem_size=EP,
                        )
                        g_hi = ghi_pool.tile([P, t_hi * EP], bf16, tag="ghi")
                        nc.gpsimd.dma_gather(
                            out_ap=g_hi[:].rearrange("p (t e) -> p t e", e=EP),
                            in_ap=table_hi,
                            idxs_ap=idx_hi_t[:, c * t_hi * 8:(c + 1) * t_hi * 8],
                            num_idxs=t_hi * P,
                            num_idxs_reg=t_hi * P,
                            elem_size=EP,
                        )
                    else:
                        g_lo = glo_pool.tile([P, t_lo * EP], bf16, tag="glo")
                        g_hi = ghi_pool.tile([P, t_hi * EP], bf16, tag="ghi")
                    if ablate == "gather":
                        continue
                    m_ps = ps_m.tile([P, F], f32, tag="m")
                    for t in range(ntile):
                        s_t = sel_pool.tile([P, P], bf16, tag="s")
                        nc.vector.tensor_tensor(
                            out=s_t[:],
                            in0=rel_t[:, c * ntile + t: c * ntile + t + 1]
                                .to_broadcast([P, P]),
                            in1=iota_t[:],
                            op=mybir.AluOpType.is_equal,
                        )
                        g = g_lo if t < t_lo else g_hi
                        tt = t if t < t_lo else t - t_lo
                        nc.tensor.matmul(
                            out=m_ps[:], lhsT=s_t[:],
                            rhs=g[:, tt * EP: (tt + 1) * EP],
                            start=(t == 0), stop=(t == ntile - 1),
                        )
                    # transform chunk: y = relu(m @ W + b)
                    m_sb = work.tile([P, F], bf16, tag="m_sb")
                    nc.scalar.copy(m_sb[:], m_ps[:])
                    y_ps = ps_y.tile([P, F], f32, tag="y")
                    for v in range(V):
                        pt = ps_t.tile([P, P], bf16, tag="pt")
                        nc.tensor.transpose(
                            pt[:], m_sb[:, v * P:(v + 1) * P], ident[:])
                        mT = work.tile([P, P], bf16, tag="mT")
                        nc.scalar.copy(mT[:], pt[:])
                        rhs = (w1_t[:, v * P:(v + 1) * P] if rnd == 1
                               else w2_t[:])
                        nc.tensor.matmul(
                            out=y_ps[:, v * P:(v + 1) * P], lhsT=mT[:],
                            rhs=rhs, start=True, stop=True,
                        )
                    if rnd == 1:
                        y_pk = work.tile([P, EP], bf16, tag="y_pk")
                        if zero_bias:
                            nc.scalar.activation(
                                y_pk[:], y_ps[:],
                                mybir.ActivationFunctionType.Relu)
                        else:
                            y_f = work.tile([P, F], f32, tag="y_f")
                            nc.vector.tensor_add(y_f[:], y_ps[:], b1_t[:])
                            nc.scalar.activation(
                                y_pk[:], y_f[:],
                                mybir.ActivationFunctionType.Relu)
                        if c < CHUNK_A:
                            nc.sync.dma_start(
                                ya[c * P:(c + 1) * P, :], y_pk[:])
                        else:
                            nc.sync.dma_start(
                                yb[(c - CHUNK_A) * P:(c - CHUNK_A + 1) * P, :],
                                y_pk[:])
                    else:
                        y_sb = work.tile([P, F], f32, tag="y_sb")
                        if zero_bias:
                            nc.scalar.activation(
                                y_sb[:], y_ps[:],
                                mybir.ActivationFunctionType.Relu)
                        else:
                            nc.vector.tensor_add(y_sb[:], y_ps[:], b2_t[:])
                            nc.scalar.activation(
                                y_sb[:], y_sb[:],
                                mybir.ActivationFunctionType.Relu)
                        nc.sync.dma_start(
                            z_out[c * P:(c + 1) * P, :], y_sb[:])

            for _rep in range(repeat):
                y_sh_a = dram.tile([ROWS_A, EP], bf16, tag=f"ysa{_rep}")
                y_sh_b = dram.tile([ROWS_B, EP], bf16, tag=f"ysb{_rep}")
                y_full_a = dram.tile([NCORES * ROWS_A, EP], bf16,
                                     addr_space="Shared", tag=f"yfa{_rep}")
                y_full_b = dram.tile([NCORES * ROWS_B, EP], bf16,
                                     addr_space="Shared", tag=f"yfb{_rep}")
                do_round(1, t_lo1, t_hi1, idx_tiles["1lo"], idx_tiles["1hi"],
                         rel1_t, h_pk[:SPLIT], h_pk[SPLIT:], y_sh_a, y_sh_b)

                if not sim_single and ablate != "gather":
                    nc.gpsimd.collective_compute(
                        "AllGather", mybir.AluOpType.bypass,
                        replica_groups=[list(range(NCORES))],
                        ins=[y_sh_a[:]], outs=[y_full_a[:]],
                    )
                    nc.gpsimd.collective_compute(
                        "AllGather", mybir.AluOpType.bypass,
                        replica_groups=[list(range(NCORES))],
                        ins=[y_sh_b[:]], outs=[y_full_b[:]],
                    )

                do_round(2, t_lo2, t_hi2, idx_tiles["2lo"], idx_tiles["2hi"],
                         rel2_t, y_full_a[:], y_full_b[:], None, None)

    nc.compile()
    _BUILD_CACHE[key] = nc
    return nc


def prep_inputs(h, src, dst, W1, b1, W2, b2):
    h = np.asarray(h, np.float32)
    src = np.asarray(src).astype(np.int64)
    dst = np.asarray(dst).astype(np.int64)
    W1 = np.asarray(W1, np.float32)
    b1 = np.asarray(b1, np.float32)
    W2 = np.asarray(W2, np.float32)
    b2 = np.asarray(b2, np.float32)

    # ---- host prep: index tables (integer metadata only) ----
    # round 1: gather key = src id (table = packed h split at 32768)
    lo1 = src < SPLIT
    w_lo1 = np.bincount(dst[lo1], minlength=N_NODES)
    w_hi1 = np.bincount(dst[~lo1], minlength=N_NODES)
    p1, t_lo1, t_hi1 = _pack_bins(w_lo1, w_hi1, 6 * P, 3 * P)
    t_lo1, t_hi1 = max(t_lo1, 6), max(t_hi1, 3)

    # round 2: gather key = y-table row of src. The y table is AllGathered in
    # two pieces: rows [0, ROWS_A) of each core (concat -> 32768 rows = lo
    # table), rows [ROWS_A, SLOTS) (concat -> 18432 rows = hi table).
    core1 = p1 // SLOTS
    row1 = p1 % SLOTS
    ytab = np.where(row1 < ROWS_A,
                    core1 * ROWS_A + row1,
                    SPLIT + core1 * ROWS_B + (row1 - ROWS_A))
    key2 = ytab[src]
    lo2 = key2 < SPLIT
    w_lo2 = np.bincount(dst[lo2], minlength=N_NODES)
    w_hi2 = np.bincount(dst[~lo2], minlength=N_NODES)
    p2, t_lo2, t_hi2 = _pack_bins(w_lo2, w_hi2, 6 * P, 3 * P)
    t_lo2, t_hi2 = max(t_lo2, 6), max(t_hi2, 3)

    i1l, i1h, r1 = _edge_tables(src, p1[dst], t_lo1, t_hi1)
    i2l, i2h, r2 = _edge_tables(key2, p2[dst], t_lo2, t_hi2)

    i1l, i1h = _idx_layout(i1l), _idx_layout(i1h)
    i2l, i2h = _idx_layout(i2l), _idx_layout(i2h)
    r1, r2 = _rel_layout(r1), _rel_layout(r2)

    h_pk = h.astype(ml_dtypes.bfloat16)
    b1_flat = b1.reshape(F)
    b2_flat = np.tile(b2, V)
    zero_bias = not (b1_flat.any() or b2_flat.any())
    b1_rep = np.broadcast_to(b1_flat, (P, F)).copy()
    b2_rep = np.broadcast_to(b2_flat, (P, F)).copy()
    iota = np.broadcast_to(np.arange(P, dtype=np.float32), (P, P)).copy()
    w1_b = W1.astype(ml_dtypes.bfloat16)
    w2_b = W2.astype(ml_dtypes.bfloat16)

    in_maps = []
    for c in range(NCORES):
        in_maps.append({
            "h_pk": h_pk, "w1": w1_b, "w2": w2_b, "b1r": b1_rep, "b2r": b2_rep,
            "iota": iota,
            "idx1_lo": i1l[c], "idx1_hi": i1h[c],
            "idx2_lo": i2l[c], "idx2_hi": i2h[c],
            "rel1": r1[c], "rel2": r2[c],
        })

    return {
        "in_maps": in_maps,
        "tvals": (t_lo1, t_hi1, t_lo2, t_hi2),
        "zero_bias": zero_bias,
        "p2": p2,
    }


LAST_RESULT = None


def kernel(h, src, dst, W1, b1, W2, b2, _trace=False, _tmpdir=None):
    global LAST_RESULT
    prep = prep_inputs(h, src, dst, W1, b1, W2, b2)
    nc = _build(*prep["tvals"], zero_bias=prep["zero_bias"])
    res = bass_utils.run_bass_kernel_spmd(
        nc, prep["in_maps"], core_ids=list(range(NCORES)),
        trace=_trace, tmpdir=_tmpdir,
    )
    LAST_RESULT = res
    z_full = np.concatenate([res.results[c]["z_out"] for c in range(NCORES)],
                            axis=0)
    return z_full[prep["p2"]].astype(np.float32)
